# revision 8
# baseline (speedup 1.0000x reference)
"""Trainium2 Bass kernel for nn_CrossAttention (dense transformer cross-attention).

Sharding: 8 cores = 2 (batch DP) x 4 (head-group TP, 4 heads each).
Each core computes, for its (batch, 4-head group):
  LN(x), LN(context) folded into projections; q/k in transposed layout,
  v in normal layout; scores^T = k^T.T-style PE matmuls (2-head row packing);
  softmax via exp(scores)*exp(alibi) factorization (exp(alibi) precomputed on
  host); attn@v with an appended ones-column producing row sums (softmax
  denominators) for free; normalization via ACT ln/exp reciprocal; output
  projection against the core's Wo row-slice -> partial output, summed on host.

All matmuls bf16 with fp32 PSUM accumulation.
"""

import numpy as np
import ml_dtypes

import concourse.bass as bass
import concourse.tile as tile
import concourse.mybir as mybir
from concourse import bacc
from concourse.bass_utils import run_bass_kernel_spmd

BF16 = ml_dtypes.bfloat16
F32 = mybir.dt.float32
BF = mybir.dt.bfloat16

B = 2
N = 2048          # query tokens
M = 2048          # context tokens
DIM = 1024
HEADS = 16
DH = 64
HPC = 4           # heads per core
DHC = HPC * DH    # 256 head-dims per core
SCALE = DH ** -0.5
LN_EPS = 1e-5
N_CORES = 8

NT = N // 512     # 4 i-chunks
JT = M // 128     # 16 j-tiles
DT = DIM // 128   # 8 model-dim tiles

AluOp = mybir.AluOpType
Act = mybir.ActivationFunctionType


def build_program(with_bias_qk=False, with_bias_v=False):
    nc = bacc.Bacc("TRN2", target_bir_lowering=False, debug=False,
                   num_devices=N_CORES)

    xT = nc.dram_tensor("xT", [DIM, N], BF, kind="ExternalInput")
    cT = nc.dram_tensor("cT", [DIM, M], BF, kind="ExternalInput")
    wq = nc.dram_tensor("wq", [DIM, DHC], BF, kind="ExternalInput")
    wk = nc.dram_tensor("wk", [DIM, DHC], BF, kind="ExternalInput")
    wv = nc.dram_tensor("wv", [DIM, DHC], BF, kind="ExternalInput")
    wo = nc.dram_tensor("wo", [DHC, DIM], BF, kind="ExternalInput")
    # per-partition vectors: cols 0 = -s_q, 1 = -s_k, 2 = bq, 3 = bk
    sv = nc.dram_tensor("sv", [DHC, 4], F32, kind="ExternalInput")
    # rows: 0 = s_v, 1 = bv
    svr = nc.dram_tensor("svr", [2, DHC], F32, kind="ExternalInput")
    ea = nc.dram_tensor("ea", [HPC, M, N], BF, kind="ExternalInput")
    po = nc.dram_tensor("po", [DIM, N], F32, kind="ExternalOutput")
    scratch = nc.dram_tensor("scratch", [2, M], F32, kind="Internal")

    ea_ap = ea.ap()
    po_ap = po.ap()

    with tile.TileContext(nc) as tc:
        with (
            tc.tile_pool(name="const", bufs=1) as cpool,
            tc.tile_pool(name="xin", bufs=1) as xpool,
            tc.tile_pool(name="wts", bufs=1) as wpool,
            tc.tile_pool(name="qkv", bufs=1) as qpool,
            tc.tile_pool(name="sq", bufs=3) as sqpool,
            tc.tile_pool(name="tmp", bufs=3) as tpool,
        ):
            # ---- load persistent tensors ----
            xt = [xpool.tile([128, N], BF, tag=f"x{d}", name=f"x{d}") for d in range(DT)]
            ct = [xpool.tile([128, M], BF, tag=f"c{d}", name=f"c{d}") for d in range(DT)]
            for d in range(DT):
                nc.sync.dma_start(xt[d][:], xT.ap()[d * 128:(d + 1) * 128, :])
                nc.sync.dma_start(ct[d][:], cT.ap()[d * 128:(d + 1) * 128, :])
            wqt = [wpool.tile([128, DHC], BF, tag=f"wq{d}", name=f"wq{d}") for d in range(DT)]
            wkt = [wpool.tile([128, DHC], BF, tag=f"wk{d}", name=f"wk{d}") for d in range(DT)]
            wvt = [wpool.tile([128, DHC], BF, tag=f"wv{d}", name=f"wv{d}") for d in range(DT)]
            for d in range(DT):
                nc.sync.dma_start(wqt[d][:], wq.ap()[d * 128:(d + 1) * 128, :])
                nc.sync.dma_start(wkt[d][:], wk.ap()[d * 128:(d + 1) * 128, :])
                nc.sync.dma_start(wvt[d][:], wv.ap()[d * 128:(d + 1) * 128, :])
            wot = [wpool.tile([128, DIM], BF, tag=f"wo{k}", name=f"wo{k}") for k in range(2)]
            for k in range(2):
                nc.sync.dma_start(wot[k][:], wo.ap()[k * 128:(k + 1) * 128, :])
            svt = [cpool.tile([128, 4], F32, tag=f"sv{m}", name=f"svt{m}") for m in range(2)]
            for m in range(2):
                nc.sync.dma_start(svt[m][:], sv.ap()[m * 128:(m + 1) * 128, :])
            svrow = cpool.tile([1, DHC], F32, tag="svrow")
            nc.sync.dma_start(svrow[:], svr.ap()[0:1, :])
            sv_b = cpool.tile([128, DHC], F32, tag="sv_b")
            nc.gpsimd.partition_broadcast(sv_b[:], svrow[:])
            bv_b = None
            if with_bias_v:
                bvrow = cpool.tile([1, DHC], F32, tag="bvrow")
                nc.sync.dma_start(bvrow[:], svr.ap()[1:2, :])
                bv_b = cpool.tile([128, DHC], F32, tag="bv_b")
                nc.gpsimd.partition_broadcast(bv_b[:], bvrow[:])

            # selector lhsT for stats matmuls: [128, 66] bf16.
            # slice [0:33]: col 0 = 1/1024 -> psum row 0 (mean, divisor folded)
            # slice [33:66]: col 65 = 1/1024 -> psum row 32 (mean of squares;
            # row 32 so later single-row engine reads are 32-aligned)
            sel = cpool.tile([128, 66], BF, tag="sel")
            nc.vector.memset(sel[:], 0.0)
            nc.vector.memset(sel[:, 0:1], 1.0 / DIM)
            nc.vector.memset(sel[:, 65:66], 1.0 / DIM)

            qT = [qpool.tile([128, N], BF, tag=f"qT{p}", name=f"qT{p}")
                  for p in range(2)]
            kT = [qpool.tile([128, M], BF, tag=f"kT{p}", name=f"kT{p}")
                  for p in range(2)]
            # v in normal layout with ones columns (stride 66 per head)
            vt = [qpool.tile([128, HPC * 66], BF, tag=f"v{j}", name=f"v{j}")
                  for j in range(JT)]
            for j in range(JT):
                for h in range(HPC):
                    nc.vector.memset(vt[j][:, 66 * h + 64:66 * h + 65], 1.0)
                    nc.vector.memset(vt[j][:, 66 * h + 65:66 * h + 66], 0.0)

            # ---- phase 1: LN stats + projections + corrections, per side ----
            with tc.tile_pool(name="psum1", bufs=1, space="PSUM") as pp1, \
                 tc.tile_pool(name="psumproj", bufs=3, space="PSUM") as ppj, \
                 tc.tile_pool(name="ln", bufs=1) as lnpool:
                for side, tiles, nn_ in ((0, xt, N), (1, ct, M)):
                    stats = pp1.tile([33, nn_], F32, tag="stats",
                                     name=f"stats{side}")
                    for d in range(DT):
                        s = sqpool.tile([128, nn_], BF, tag="sqt",
                                        name=f"sq{side}_{d}")
                        nc.vector.tensor_mul(s[:], tiles[d][:], tiles[d][:])
                        for ic in range(nn_ // 512):
                            isl = bass.ts(ic, 512)
                            nc.tensor.matmul(
                                stats[:, isl], sel[:, 0:33], tiles[d][:, isl],
                                start=(d == 0), stop=False,
                                skip_group_check=True)
                            nc.tensor.matmul(
                                stats[:, isl], sel[:, 33:66], s[:, isl],
                                start=False, stop=(d == DT - 1),
                                skip_group_check=True)

                    murow = lnpool.tile([1, nn_], F32, tag="murow",
                                        name=f"murow{side}")
                    nc.scalar.copy(murow[:], stats[0:1, :])
                    msrow = tpool.tile([1, nn_], F32, tag="rowtmp", bufs=3,
                                       name=f"msrow{side}")
                    nc.scalar.copy(msrow[:], stats[32:33, :])
                    musq = tpool.tile([1, nn_], F32, tag="rowtmp", bufs=3,
                                      name=f"musq{side}")
                    nc.scalar.activation(musq[:], murow[:], Act.Square)
                    var = tpool.tile([1, nn_], F32, tag="rowtmp", bufs=3,
                                     name=f"var{side}")
                    nc.vector.scalar_tensor_tensor(
                        var[:], msrow[:], LN_EPS, musq[:],
                        AluOp.add, AluOp.subtract)
                    lnv = tpool.tile([1, nn_], F32, tag="rowtmp", bufs=3,
                                     name=f"lnv{side}")
                    nc.scalar.activation(lnv[:], var[:], Act.Ln)
                    rstd = lnpool.tile([1, nn_], F32, tag="rstd",
                                       name=f"rstd{side}")
                    nc.scalar.activation(rstd[:], lnv[:], Act.Exp, scale=-0.5)

                    mu_b = lnpool.tile([128, nn_], F32, tag="mu_b",
                                       name=f"mu_b{side}")
                    nc.gpsimd.partition_broadcast(mu_b[:], murow[:])
                    rstd_b = lnpool.tile([128, nn_], F32, tag="rstd_b",
                                         name=f"rstd_b{side}")
                    nc.gpsimd.partition_broadcast(rstd_b[:], rstd[:])

                    # projection of q (side 0) / k (side 1), transposed layout
                    wtl = wqt if side == 0 else wkt
                    dst = qT if side == 0 else kT
                    scol, bcol = (0, 2) if side == 0 else (1, 3)
                    for mt in range(2):
                        msl = bass.ts(mt, 128)
                        for ic in range(nn_ // 512):
                            isl = bass.ts(ic, 512)
                            ps = ppj.tile([128, 512], F32, tag="proj",
                                          name=f"ps{side}_{mt}_{ic}")
                            for d in range(DT):
                                nc.tensor.matmul(
                                    ps[:], wtl[d][:, msl], tiles[d][:, isl],
                                    start=(d == 0), stop=(d == DT - 1))
                            tmp = tpool.tile([128, 512], F32, tag="ctmp",
                                             name=f"ct{side}_{mt}_{ic}")
                            nc.vector.scalar_tensor_tensor(
                                tmp[:], mu_b[:, isl], svt[mt][:, scol:scol + 1],
                                ps[:], AluOp.mult, AluOp.add)
                            if with_bias_qk:
                                nc.vector.tensor_scalar_add(
                                    tmp[:], tmp[:], svt[mt][:, bcol:bcol + 1])
                            nc.vector.tensor_mul(
                                dst[mt][:, isl], tmp[:], rstd_b[:, isl])

                    if side == 1:
                        # -(mu * rstd) row, then column layouts via DRAM bounce
                        mrn = lnpool.tile([1, M], F32, tag="mrn")
                        nc.vector.scalar_tensor_tensor(
                            mrn[:], murow[:], -1.0, rstd[:],
                            AluOp.mult, AluOp.mult)
                        nc.sync.dma_start(scratch.ap()[0:1, :], rstd[:])
                        nc.sync.dma_start(scratch.ap()[1:2, :], mrn[:])
                        rstd_c_col = cpool.tile([128, JT], F32,
                                                tag="rstd_c_col")
                        nc.sync.dma_start(
                            rstd_c_col[:],
                            scratch.ap()[0, :].rearrange("(c p) -> p c", p=128))
                        mrn_c_col = cpool.tile([128, JT], F32, tag="mrn_c_col")
                        nc.sync.dma_start(
                            mrn_c_col[:],
                            scratch.ap()[1, :].rearrange("(c p) -> p c", p=128))

                        # v projection (normal layout)
                        for jc in range(JT):
                            jsl = bass.ts(jc, 128)
                            ps = ppj.tile([128, DHC], F32, tag="proj",
                                          name=f"psv{jc}")
                            for d in range(DT):
                                nc.tensor.matmul(
                                    ps[:], ct[d][:, jsl], wvt[d][:],
                                    start=(d == 0), stop=(d == DT - 1))
                            tmp = tpool.tile([128, DHC], F32, tag="vtmp",
                                             name=f"vt{jc}")
                            nc.vector.tensor_scalar_mul(
                                tmp[:], ps[:], rstd_c_col[:, jc:jc + 1])
                            vdst = vt[jc][:].rearrange(
                                "p (h c) -> p h c", c=66)[:, :, 0:64]
                            stt_in0 = sv_b[:].rearrange("p (h c) -> p h c", c=64)
                            stt_in1 = tmp[:].rearrange("p (h c) -> p h c", c=64)
                            nc.vector.scalar_tensor_tensor(
                                vdst, stt_in0, mrn_c_col[:, jc:jc + 1], stt_in1,
                                AluOp.mult, AluOp.add)
                            if with_bias_v:
                                nc.vector.tensor_add(
                                    vdst,
                                    vt[jc][:].rearrange(
                                        "p (h c) -> p h c", c=66)[:, :, 0:64],
                                    bv_b[:].rearrange("p (h c) -> p h c", c=64))

            # ---- phase 2: attention ----
            outT = [qpool.tile([128, N], BF, tag=f"oT{p}", name=f"oT{p}") for p in range(2)]
            with (
                tc.tile_pool(name="psum2", bufs=1, space="PSUM") as pp2,
                tc.tile_pool(name="ea", bufs=4) as eapool,
                tc.tile_pool(name="ex", bufs=4) as expool,
                tc.tile_pool(name="at", bufs=4) as atpool,
                tc.tile_pool(name="rp", bufs=2) as rpool,
            ):
                for pair in range(2):
                    for ic in range(NT):
                        isl = bass.ts(ic, 512)
                        ops = [pp2.tile([65, 512], F32, tag=f"out{hh}", bufs=2, name=f"ops{hh}")
                               for hh in range(2)]
                        for jt in range(JT):
                            jsl = bass.ts(jt, 128)
                            sA = pp2.tile([128, 512], F32, tag="sA", bufs=2)
                            sB = pp2.tile([128, 512], F32, tag="sB", bufs=2)
                            nc.tensor.matmul(
                                sA[:], kT[pair][0:64, jsl], qT[pair][0:64, isl],
                                start=True, stop=True)
                            nc.tensor.matmul(
                                sB[:], kT[pair][64:128, jsl], qT[pair][64:128, isl],
                                start=True, stop=True, tile_position=(64, 0))
                            for hh, sps in ((0, sA), (1, sB)):
                                h = pair * 2 + hh
                                eat = eapool.tile([128, 512], BF, tag="ea")
                                nc.sync.dma_start(
                                    eat[:],
                                    ea_ap[h, jt * 128:(jt + 1) * 128,
                                          ic * 512:(ic + 1) * 512])
                                ex = expool.tile([128, 512], BF, tag="ex")
                                nc.scalar.activation(ex[:], sps[:], Act.Exp)
                                at = atpool.tile([128, 512], BF, tag="at")
                                nc.vector.tensor_mul(at[:], ex[:], eat[:])
                                nc.tensor.matmul(
                                    ops[hh][:], vt[jt][:, 66 * h:66 * h + 65],
                                    at[:], start=(jt == 0), stop=(jt == JT - 1))
                        for hh in range(2):
                            dln = rpool.tile([1, 512], F32, tag="dln")
                            nc.scalar.activation(
                                dln[:], ops[hh][64:65, :], Act.Ln)
                            rrow = rpool.tile([1, 512], F32, tag="rrow")
                            nc.scalar.activation(
                                rrow[:], dln[:], Act.Exp, scale=-1.0)
                            rb = rpool.tile([64, 512], F32, tag="rb")
                            nc.gpsimd.partition_broadcast(rb[:], rrow[:])
                            nc.vector.tensor_mul(
                                outT[pair][64 * hh:64 * hh + 64, isl],
                                ops[hh][0:64, :], rb[:])

            # ---- phase 3: output projection ----
            with tc.tile_pool(name="psum3", bufs=4, space="PSUM") as pp3, \
                 tc.tile_pool(name="oev", bufs=4) as opool:
                for mt in range(DT):
                    msl = bass.ts(mt, 128)
                    for ic in range(NT):
                        isl = bass.ts(ic, 512)
                        ps = pp3.tile([128, 512], F32, tag="po")
                        for kt in range(2):
                            nc.tensor.matmul(
                                ps[:], wot[kt][:, msl], outT[kt][:, isl],
                                start=(kt == 0), stop=(kt == 1))
                        ot = opool.tile([128, 512], F32, tag="oev")
                        nc.scalar.copy(ot[:], ps[:])
                        nc.sync.dma_start(po_ap[mt * 128:(mt + 1) * 128, isl],
                                          ot[:])

    nc.compile()
    return nc


_cache = {}
RUN_KWARGS = {}
LAST_RESULT = None


def kernel(x, context, alibi, Wq, Wk, Wv, Wo, bo, ln_w, ln_b):
    x = np.asarray(x, np.float32)
    context = np.asarray(context, np.float32)
    alibi = np.asarray(alibi, np.float32)
    Wq = np.asarray(Wq, np.float32)
    Wk = np.asarray(Wk, np.float32)
    Wv = np.asarray(Wv, np.float32)
    Wo = np.asarray(Wo, np.float32)
    bo = np.asarray(bo, np.float32)
    ln_w = np.asarray(ln_w, np.float32)
    ln_b = np.asarray(ln_b, np.float32)

    # host-side folding
    Wq_f = Wq * ln_w[None, :] * SCALE
    Wk_f = Wk * ln_w[None, :]
    Wv_f = Wv * ln_w[None, :]
    bq = (Wq @ ln_b) * SCALE
    bk = Wk @ ln_b
    bv = Wv @ ln_b
    with_bias_qk = bool(np.any(bq) or np.any(bk))
    with_bias_v = bool(np.any(bv))

    key = (with_bias_qk, with_bias_v)
    if key not in _cache:
        _cache[key] = build_program(*key)
    nc = _cache[key]

    # per-head-group shared data
    hg_data = []
    for hg in range(4):
        hs = slice(hg * DHC, (hg + 1) * DHC)
        wq_l = np.ascontiguousarray(Wq_f[hs].T).astype(BF16)
        wk_l = np.ascontiguousarray(Wk_f[hs].T).astype(BF16)
        wv_l = np.ascontiguousarray(Wv_f[hs].T).astype(BF16)
        wo_l = np.ascontiguousarray(Wo[:, hs].T).astype(BF16)
        s_q = wq_l.astype(np.float32).sum(axis=0)
        s_k = wk_l.astype(np.float32).sum(axis=0)
        s_v = wv_l.astype(np.float32).sum(axis=0)
        svec = np.stack([-s_q, -s_k, bq[hs], bk[hs]], axis=1).astype(np.float32)
        svrow = np.stack([s_v, bv[hs]], axis=0).astype(np.float32)
        eah = np.exp(alibi[hg * HPC:(hg + 1) * HPC])
        eah = np.ascontiguousarray(eah.transpose(0, 2, 1)).astype(BF16)
        hg_data.append((wq_l, wk_l, wv_l, wo_l, svec, svrow, eah))

    xTb = [np.ascontiguousarray(x[bi].T).astype(BF16) for bi in range(B)]
    cTb = [np.ascontiguousarray(context[bi].T).astype(BF16) for bi in range(B)]

    in_maps = []
    for c in range(N_CORES):
        bi, hg = c // 4, c % 4
        wq_l, wk_l, wv_l, wo_l, svec, svrow, eah = hg_data[hg]
        in_maps.append({
            "xT": xTb[bi], "cT": cTb[bi],
            "wq": wq_l, "wk": wk_l, "wv": wv_l, "wo": wo_l,
            "sv": svec, "svr": svrow, "ea": eah,
        })

    res = run_bass_kernel_spmd(nc, in_maps, core_ids=list(range(N_CORES)),
                               **RUN_KWARGS)
    globals()['LAST_RESULT'] = res

    out = np.zeros((B, N, DIM), np.float32)
    for c in range(N_CORES):
        bi = c // 4
        out[bi] += res.results[c]["po"].T
    out += bo[None, None, :]
    return out
